# Initial kernel scaffold
#
"""Bahdanau-attention kernel for Trainium2 (8 NeuronCores, data-parallel over batch).

Computes, for each batch b:
    q[b]    = v * (W_w @ prev[b] + W_b + U_b)            (host, tiny)
    U'      = v[:, None] * U_w                            (host, tiny)
    e[b,t]  = sum_h relu(q[b,h] + (U' @ enc[b,t])_h)      (device)
    alpha   = softmax(e[b, :])                            (device)
    out[b]  = sum_t alpha[t] * enc[b,t,:]                 (device)

The v>0 fold is exact: v_h * relu(x_h) == relu(v_h * x_h) for v_h >= 0.

Device strategy (per core: 4 batches, enc slice [4, 4096, 1024] fp32 = 64 MB
streamed from HBM exactly once):
  - enc tiles [t=128, c=1024] stay SBUF-resident for the batch.
  - PE transposes each tile chunk-wise to [c, t] (float32r, PSUM), copied to
    SBUF alternating DVE/ACT.
  - U-matmul in float32r accumulates [t=128, h=256] in PSUM on top of a
    ones-row x q bias matmul.
  - ACT fused relu+row-reduce produces the energy column per tile.
  - Exact fp32 softmax over the batch (GPSIMD cross-partition all-reduce).
  - Pass-2 weighted sum: alpha column as stationary, natural enc tile as
    moving operand, accumulated into PSUM [1, 1024].
"""

import os
import sys

import numpy as np

sys.path.insert(0, "/opt/trn_rl_repo")

import concourse.bass as bass
import concourse.bass_isa as bass_isa
import concourse.mybir as mybir
import concourse.tile as tile
from concourse.bass import ts
from concourse.bass_utils import run_bass_kernel_spmd
from concourse.masks import make_identity

B, T, C, H, D = 32, 4096, 1024, 256, 512
NCORES = 8
BPC = B // NCORES  # batches per core

F32 = mybir.dt.float32
F32R = mybir.dt.float32r
BF16 = mybir.dt.bfloat16

P = 128            # partitions / t-tile size
CK = C // P        # 8 c-chunks per tile
NT = T // P        # 32 t-tiles per batch


def build_bass(bpc: int = BPC, n_tiles: int = NT):
    nc = bass.Bass(target_bir_lowering=False, trn_type="TRN2")

    enc = nc.dram_tensor("enc", [bpc, n_tiles * P, C], F32, kind="ExternalInput")
    # q rows packed on one partition: [1, bpc*H]
    qrow = nc.dram_tensor("qrow", [1, bpc * H], F32, kind="ExternalInput")
    # U' transposed, pre-arranged host-side as [p, chunk, h] with c = chunk*128 + p
    ut = nc.dram_tensor("ut", [P, CK, H], F32, kind="ExternalInput")
    out = nc.dram_tensor("out", [bpc, C], F32, kind="ExternalOutput")

    enc_ap = enc.ap()
    out_ap = out.ap()

    with tile.TileContext(nc) as tc:
        with (
            tc.tile_pool(name="singles", bufs=1) as singles,
            tc.tile_pool(name="enc_pool", bufs=n_tiles + 4) as enc_pool,
            tc.tile_pool(name="encT_pool", bufs=3) as encT_pool,
            tc.tile_pool(name="relu_pool", bufs=3) as relu_pool,
            tc.tile_pool(name="batch_pool", bufs=2) as batch_pool,
            tc.tile_pool(name="small_pool", bufs=8) as small_pool,
            tc.tile_pool(name="outst_pool", bufs=2) as outst_pool,
            tc.tile_pool(name="ps_tp", bufs=2, space="PSUM") as ps_tp,
            tc.tile_pool(name="ps_um", bufs=2, space="PSUM") as ps_um,
            tc.tile_pool(name="ps_c", bufs=1, space="PSUM") as ps_c,
        ):
            identity = singles.tile([P, P], F32R)
            make_identity(nc, identity)
            ut_s = singles.tile([P, CK, H], F32)
            nc.sync.dma_start(out=ut_s, in_=ut.ap())
            q_s = singles.tile([1, bpc * H], F32)
            nc.sync.dma_start(out=q_s, in_=qrow.ap())
            ones_row = singles.tile([1, P], F32)
            nc.vector.memset(ones_row, 1.0)

            for b in range(bpc):
                # ---------------- pass 1: energies ----------------
                enc_tiles = []
                e_buf = batch_pool.tile([P, n_tiles], F32, tag="ebuf")
                for j in range(n_tiles):
                    enc_t = enc_pool.tile([P, C], F32, tag="enc")
                    nc.sync.dma_start(out=enc_t, in_=enc_ap[b, ts(j, P), :])
                    enc_tiles.append(enc_t)

                    # transpose 8 chunks: [t,c]->[c,t] into PSUM (f32r)
                    tp = ps_tp.tile([P, C], F32R, tag="tp")
                    for k in range(CK):
                        nc.tensor.transpose(
                            tp[:, ts(k, P)],
                            enc_t.bitcast(F32R)[:, ts(k, P)],
                            identity,
                        )
                    encT = encT_pool.tile([P, C], F32, tag="encT")
                    if j % 2 == 0:
                        nc.vector.tensor_copy(encT, tp.bitcast(F32))
                    else:
                        nc.scalar.copy(encT, tp.bitcast(F32))

                    # U-matmul: psum[t, h] = q[h] + sum_c encT[c,t]^T ut[c,h]
                    um = ps_um.tile([P, H], F32, tag="um")
                    nc.tensor.matmul(
                        um,
                        ones_row.bitcast(F32R),
                        q_s.bitcast(F32R)[:, b * H : (b + 1) * H],
                        start=True,
                        stop=False,
                    )
                    for k in range(CK):
                        nc.tensor.matmul(
                            um,
                            encT.bitcast(F32R)[:, ts(k, P)],
                            ut_s.bitcast(F32R)[:, k, :],
                            start=False,
                            stop=(k == CK - 1),
                        )

                    # e[t] = sum_h relu(um[t, h])
                    relu_sc = relu_pool.tile([P, H], BF16, tag="relu")
                    nc.scalar.activation(
                        out=relu_sc,
                        in_=um,
                        func=mybir.ActivationFunctionType.Relu,
                        accum_out=e_buf[:, j : j + 1],
                    )

                # ---------------- softmax (exact, fp32) ----------------
                mp = small_pool.tile([P, 1], F32, tag="mp")
                nc.vector.tensor_reduce(
                    mp, e_buf, axis=mybir.AxisListType.X, op=mybir.AluOpType.max
                )
                mall = small_pool.tile([P, 1], F32, tag="mall")
                nc.gpsimd.partition_all_reduce(
                    mall, mp, channels=P, reduce_op=bass_isa.ReduceOp.max
                )
                mneg = small_pool.tile([P, 1], F32, tag="mneg")
                nc.vector.tensor_scalar_mul(mneg, mall, -1.0)
                z = batch_pool.tile([P, n_tiles], F32, tag="z")
                s = small_pool.tile([P, 1], F32, tag="s")
                nc.scalar.activation(
                    out=z,
                    in_=e_buf,
                    func=mybir.ActivationFunctionType.Exp,
                    bias=mneg,
                    accum_out=s,
                )
                sall = small_pool.tile([P, 1], F32, tag="sall")
                nc.gpsimd.partition_all_reduce(
                    sall, s, channels=P, reduce_op=bass_isa.ReduceOp.add
                )
                rec = small_pool.tile([P, 1], F32, tag="rec")
                nc.vector.reciprocal(rec, sall)
                alpha = batch_pool.tile([P, n_tiles], F32, tag="alpha")
                nc.vector.tensor_scalar_mul(alpha, z, rec)

                # ---------------- pass 2: weighted sum ----------------
                cps = ps_c.tile([1, 2, D], F32, tag="cps")
                for j in range(n_tiles):
                    for h in range(2):
                        nc.tensor.matmul(
                            cps[:, h, :],
                            alpha.bitcast(F32R)[:, j : j + 1],
                            enc_tiles[j].bitcast(F32R)[:, ts(h, D)],
                            start=(j == 0),
                            stop=(j == n_tiles - 1),
                        )
                c_st = outst_pool.tile([1, C], F32, tag="cst")
                nc.vector.tensor_copy(c_st, cps.rearrange("p a b -> p (a b)"))
                nc.sync.dma_start(out=out_ap[b : b + 1, :], in_=c_st)

    return nc


_NC_CACHE: dict = {}


def _get_nc(bpc=BPC, n_tiles=NT):
    key = (bpc, n_tiles)
    if key not in _NC_CACHE:
        _NC_CACHE[key] = build_bass(bpc, n_tiles)
    return _NC_CACHE[key]


def _host_prep(previous_decoder_hidden_state, W_w, W_b, U_w, U_b, v):
    prev = np.asarray(previous_decoder_hidden_state, dtype=np.float32)[:, 0, :]
    W_w = np.asarray(W_w, dtype=np.float32)
    U_w = np.asarray(U_w, dtype=np.float32)
    v = np.asarray(v, dtype=np.float32)
    bias = np.asarray(W_b, dtype=np.float32) + np.asarray(U_b, dtype=np.float32)
    q_all = (v[None, :] * (prev @ W_w.T + bias)).astype(np.float32)  # [B, H]
    up = (v[:, None] * U_w).astype(np.float32)  # [H, C]
    # ut_host[p, k, h] = up.T[k*128 + p, h]
    ut_host = np.ascontiguousarray(up.T.reshape(CK, P, H).transpose(1, 0, 2))
    return q_all, ut_host


def kernel(**inputs) -> np.ndarray:
    enc = np.ascontiguousarray(
        np.asarray(inputs["encoder_final_hidden_layers"], dtype=np.float32)
    )
    q_all, ut_host = _host_prep(
        inputs["previous_decoder_hidden_state"],
        inputs["W_w"],
        inputs["W_b"],
        inputs["U_w"],
        inputs["U_b"],
        inputs["v"],
    )

    nc = _get_nc()
    in_maps = []
    for i in range(NCORES):
        sl = slice(i * BPC, (i + 1) * BPC)
        in_maps.append(
            {
                "enc": enc[sl],
                "qrow": np.ascontiguousarray(q_all[sl].reshape(1, BPC * H)),
                "ut": ut_host,
            }
        )
    res = run_bass_kernel_spmd(nc, in_maps, core_ids=list(range(NCORES)))
    return np.concatenate([r["out"] for r in res.results], axis=0)


if __name__ == "__main__":
    nc = build_bass()
    print("built ok")


# revision 26
# speedup vs baseline: 1.0409x; 1.0409x over previous
"""Bahdanau-attention kernel for Trainium2 (8 NeuronCores, data-parallel over batch).

Computes, for each batch b:
    q[b]    = v * (W_w @ prev[b] + W_b + U_b)            (host, tiny)
    U'      = v[:, None] * U_w                            (host, tiny)
    e[b,t]  = sum_h relu(q[b,h] + (U' @ enc[b,t])_h)      (device)
    alpha   = softmax(e[b, :])                            (device)
    out[b]  = sum_t alpha[t] * enc[b,t,:]                 (device)

The v>0 fold is exact: v_h * relu(x_h) == relu(v_h * x_h) for v_h >= 0.

Device strategy (per core: 4 batches, enc slice [4, 4096, 1024] fp32 = 64 MB
streamed from HBM exactly once):
  - enc tiles [t=128, c=1024] stay SBUF-resident for the batch.
  - PE transposes each tile chunk-wise to [c, t] (float32r, PSUM), DVE copies
    the result to SBUF.
  - U-matmul in float32r accumulates [t=128, h=256] in PSUM on top of a
    ones-row x q bias matmul.
  - ACT fused relu+row-reduce produces the energy column per tile.
  - Exact fp32 softmax over the batch (PE transpose / ones-matmuls for the
    cross-partition max, sum, and broadcasts).
  - Pass-2 weighted sum: alpha column as stationary, natural enc tile as
    moving operand, accumulated into PSUM [1, 1024].

Scheduling constraint: walrus allows only one semaphore wait on the LDWEIGHTS
half of a 4-byte (fp32/f32r) matmul. The structure below keeps every PE
instruction's *new* cross-engine dependencies on a single semaphore: constants
funnel through DVE copies, a dummy PE matmul observes the DVE clock up front,
PSUM-slot release readers are DVE for the transpose/wsum pools and ACT only
for the U-matmul pool (whose sole new wait is that release).
"""

import os
import sys

import numpy as np

sys.path.insert(0, "/opt/trn_rl_repo")

import concourse.bass as bass
import concourse.bacc as bacc
import concourse.mybir as mybir
import concourse.tile as tile
from concourse.bass import ts
from concourse.bass_utils import run_bass_kernel_spmd
from concourse.masks import make_identity
from concourse.tile import add_dep_helper

B, T, C, H, D = 32, 4096, 1024, 256, 512
NCORES = 8
BPC = B // NCORES  # batches per core

F32 = mybir.dt.float32
F32R = mybir.dt.float32r
BF16 = mybir.dt.bfloat16

P = 128            # partitions / t-tile size
CK = C // P        # 8 c-chunks per tile
NT = T // P        # 32 t-tiles per batch


def build_bass(bpc: int = BPC, n_tiles: int = NT, repeat: int = 1):
    nc = bacc.Bacc(target_bir_lowering=False, trn_type="TRN2")

    enc = nc.dram_tensor("enc", [bpc, n_tiles * P, C], F32R, kind="ExternalInput")
    # q rows packed on one partition: [1, bpc*H]
    qrow = nc.dram_tensor("qrow", [1, bpc * H], F32R, kind="ExternalInput")
    # U' transposed, pre-arranged host-side as [p, chunk, h] with c = chunk*128 + p
    ut = nc.dram_tensor("ut", [P, CK, H], F32R, kind="ExternalInput")
    out = nc.dram_tensor("out", [bpc, C], F32, kind="ExternalOutput")

    enc_ap = enc.ap()
    out_ap = out.ap()

    with tile.TileContext(nc) as tc:
        with (
            tc.tile_pool(name="singles", bufs=1) as singles,
            tc.tile_pool(name="enc_pool", bufs=n_tiles + 2) as enc_pool,
            tc.tile_pool(name="encT_pool", bufs=3) as encT_pool,
            tc.tile_pool(name="relu_pool", bufs=3) as relu_pool,
            tc.tile_pool(name="batch_pool", bufs=2) as batch_pool,
            tc.tile_pool(name="small_pool", bufs=2) as small_pool,
            tc.tile_pool(name="outst_pool", bufs=2) as outst_pool,
            tc.tile_pool(name="ps_tp", bufs=2, space="PSUM") as ps_tp,
            tc.tile_pool(name="ps_um", bufs=2, space="PSUM") as ps_um,
            tc.tile_pool(name="ps_c", bufs=2, space="PSUM") as ps_c,
        ):
            # --- constants, all funneled through DVE so PE sees one clock ---
            ident_stage = singles.tile([P, P], F32)
            make_identity(nc, ident_stage)
            ut_stage = singles.tile([P, CK, H], F32)
            nc.sync.dma_start(out=ut_stage, in_=ut.ap().bitcast(F32))
            q_stage = singles.tile([1, bpc * H], F32)
            nc.sync.dma_start(out=q_stage, in_=qrow.ap().bitcast(F32))

            ones_row_f = singles.tile([1, P], F32)
            nc.vector.memset(ones_row_f, 1.0)
            ones_row = ones_row_f.bitcast(F32R)
            q_s = singles.tile([1, bpc * H], F32R)
            nc.vector.tensor_copy(q_s, q_stage)
            ut_s = singles.tile([P, CK, H], F32R)
            nc.vector.tensor_copy(ut_s, ut_stage)
            identity = singles.tile([P, P], F32R)
            nc.vector.tensor_copy(identity, ident_stage)

            def batches():
              for b in range(bpc):
                # ---------------- pass 1: energies ----------------
                enc_tiles = []
                e_buf = batch_pool.tile([P, n_tiles], F32, tag="ebuf")
                for j in range(n_tiles):
                    enc_t = enc_pool.tile([P, C], F32R, tag="enc")
                    nc.sync.dma_start(out=enc_t, in_=enc_ap[b, ts(j, P), :])
                    enc_tiles.append(enc_t)

                    # transpose per half: 4 chunks [t,c]->[c,t] into PSUM,
                    # then one DVE copy [128, 512] to SBUF.
                    # tpB is allocated second (newest release tick); a tiny
                    # spacer transpose into it absorbs the PSUM-slot release
                    # wait so the first enc-reading transpose carries only
                    # its DMA wait (walrus LDW allows a single wait).
                    encT = encT_pool.tile([P, C], F32R, tag="encT")
                    tpA = ps_tp.tile([P, D], F32R, tag="tp")
                    tpB = ps_tp.tile([P, D], F32R, tag="tp")
                    for k in range(4):
                        nc.tensor.transpose(
                            tpA[:, ts(k, P)], enc_t[:, ts(k, P)], identity
                        )
                    nc.vector.tensor_copy(encT[:, ts(0, D)], tpA)
                    for k in range(4):
                        nc.tensor.transpose(
                            tpB[:, ts(k, P)], enc_t[:, ts(4 + k, P)], identity
                        )
                    nc.vector.tensor_copy(encT[:, ts(1, D)], tpB)

                    # U-matmul: psum[t, h] = q[h] + sum_c encT[c,t]^T ut[c,h]
                    um = ps_um.tile([P, H], F32, tag="um")
                    nc.tensor.matmul(
                        um,
                        ones_row,
                        q_s[:, b * H : (b + 1) * H],
                        start=True,
                        stop=False,
                    )
                    for k in range(CK):
                        nc.tensor.matmul(
                            um,
                            encT[:, ts(k, P)],
                            ut_s[:, k, :],
                            start=False,
                            stop=(k == CK - 1),
                        )

                    # e[t] = sum_h relu(um[t, h])  (ACT, fused reduce)
                    relu_sc = relu_pool.tile([P, H], BF16, tag="relu")
                    nc.scalar.activation(
                        out=relu_sc,
                        in_=um,
                        func=mybir.ActivationFunctionType.Relu,
                        accum_out=e_buf[:, j : j + 1],
                    )

                # ------------- softmax (exact fp32, two-level, no PE) -------------
                # z'[p,j] = exp(e[p,j] - mp[p]) with the per-partition max mp
                # (ACT bias is per-partition, so no broadcast needed), then a
                # one-partition fixup computes g[p] = exp(mp[p]-M)/S and
                # alpha = z' * g  ==  exp(e-M)/S exactly.
                mp = small_pool.tile([P, 1], F32, tag="mp")
                nc.vector.tensor_reduce(
                    mp, e_buf, axis=mybir.AxisListType.X, op=mybir.AluOpType.max
                )
                mpneg = small_pool.tile([P, 1], F32, tag="mpneg")
                nc.vector.tensor_scalar_mul(mpneg, mp, -1.0)
                z = batch_pool.tile([P, n_tiles], F32, tag="z")
                s = small_pool.tile([P, 1], F32, tag="s")
                nc.scalar.activation(
                    out=z,
                    in_=e_buf,
                    func=mybir.ActivationFunctionType.Exp,
                    bias=mpneg,
                    accum_out=s,
                )
                # gather mp and s onto partition 0 (SBUF->SBUF strided DMA)
                mrow = small_pool.tile([1, P], F32, tag="mrow")
                nc.sync.dma_start(out=mrow, in_=mp)
                srow = small_pool.tile([1, P], F32, tag="srow")
                nc.sync.dma_start(out=srow, in_=s)
                mtot = small_pool.tile([1, 1], F32, tag="mtot")
                nc.vector.tensor_reduce(
                    mtot, mrow, axis=mybir.AxisListType.X, op=mybir.AluOpType.max
                )
                mtneg = small_pool.tile([1, 1], F32, tag="mtneg")
                nc.vector.tensor_scalar_mul(mtneg, mtot, -1.0)
                grow = small_pool.tile([1, P], F32, tag="grow")
                nc.scalar.activation(
                    out=grow,
                    in_=mrow,
                    func=mybir.ActivationFunctionType.Exp,
                    bias=mtneg,
                )
                wrow = small_pool.tile([1, P], F32, tag="wrow")
                nc.vector.tensor_mul(wrow, grow, srow)
                stot = small_pool.tile([1, 1], F32, tag="stot")
                nc.vector.tensor_reduce(
                    stot, wrow, axis=mybir.AxisListType.X, op=mybir.AluOpType.add
                )
                rec = small_pool.tile([1, 1], F32, tag="rec")
                nc.vector.reciprocal(rec, stot)
                gsrow = small_pool.tile([1, P], F32, tag="gsrow")
                nc.vector.tensor_scalar_mul(gsrow, grow, rec)
                # scatter g[p]/S back to one element per partition
                gscol = small_pool.tile([P, 1], F32, tag="gscol")
                nc.sync.dma_start(out=gscol, in_=gsrow)
                alpha = batch_pool.tile([P, n_tiles], F32R, tag="alpha")
                nc.vector.tensor_scalar_mul(alpha, z, gscol)

                # ---------------- pass 2: weighted sum ----------------
                cps = ps_c.tile([1, 2, D], F32, tag="cps")
                for j in range(n_tiles):
                    for h in range(2):
                        nc.tensor.matmul(
                            cps[:, h, :],
                            alpha[:, j : j + 1],
                            enc_tiles[j][:, ts(h, D)],
                            start=(j == 0),
                            stop=(j == n_tiles - 1),
                        )
                c_st = outst_pool.tile([1, C], F32, tag="cst")
                nc.vector.tensor_copy(c_st, cps.rearrange("p a b -> p (a b)"))
                nc.sync.dma_start(out=out_ap[b : b + 1, :], in_=c_st)

            if repeat == 1:
                batches()
            else:
                with tc.For_i(0, repeat, 1):
                    batches()

    return nc


_NC_CACHE: dict = {}


def _get_nc(bpc=BPC, n_tiles=NT):
    key = (bpc, n_tiles)
    if key not in _NC_CACHE:
        nc = build_bass(bpc, n_tiles)
        if not nc.is_finalized():
            nc.finalize()
        _NC_CACHE[key] = nc
    return _NC_CACHE[key]


def _host_prep(previous_decoder_hidden_state, W_w, W_b, U_w, U_b, v):
    prev = np.asarray(previous_decoder_hidden_state, dtype=np.float32)[:, 0, :]
    W_w = np.asarray(W_w, dtype=np.float32)
    U_w = np.asarray(U_w, dtype=np.float32)
    v = np.asarray(v, dtype=np.float32)
    bias = np.asarray(W_b, dtype=np.float32) + np.asarray(U_b, dtype=np.float32)
    q_all = (v[None, :] * (prev @ W_w.T + bias)).astype(np.float32)  # [B, H]
    up = (v[:, None] * U_w).astype(np.float32)  # [H, C]
    # ut_host[p, k, h] = up.T[k*128 + p, h]
    ut_host = np.ascontiguousarray(up.T.reshape(CK, P, H).transpose(1, 0, 2))
    return q_all, ut_host


def kernel(**inputs) -> np.ndarray:
    enc = np.ascontiguousarray(
        np.asarray(inputs["encoder_final_hidden_layers"], dtype=np.float32)
    )
    q_all, ut_host = _host_prep(
        inputs["previous_decoder_hidden_state"],
        inputs["W_w"],
        inputs["W_b"],
        inputs["U_w"],
        inputs["U_b"],
        inputs["v"],
    )

    nc = _get_nc()
    in_maps = []
    for i in range(NCORES):
        sl = slice(i * BPC, (i + 1) * BPC)
        in_maps.append(
            {
                "enc": enc[sl],
                "qrow": np.ascontiguousarray(q_all[sl].reshape(1, BPC * H)),
                "ut": ut_host,
            }
        )
    res = run_bass_kernel_spmd(nc, in_maps, core_ids=list(range(NCORES)))
    return np.concatenate([r["out"] for r in res.results], axis=0)


if __name__ == "__main__":
    nc = build_bass()
    print("built ok")
